# revision 1
# baseline (speedup 1.0000x reference)
"""Self-contained Trainium2 Bass kernel for nn_MultiHeadAttention_80942953660675.

Reference computation (B=2, T=2048, D=1024, H=16, hd=64, causal):
    q = x @ wq.T; k = x @ wk.T; v = x @ wv.T            (per-head split)
    out = softmax(q k^T / sqrt(hd) + causal_mask) v      (per batch, head)
    out = concat_heads(out) @ wo.T + bo

Sharding over 8 NeuronCores: core = (batch b, head-group g), b in {0,1},
g in {0..3}, each group = 4 heads (256 channels). wq/wk/wv column-sharded,
wo row-sharded (Megatron); host sums the 4 partial outputs per batch and
adds the bias.

Per-core kernel (all matmuls in float32r = full PE rate at N>=256):
  - host passes x[b].T so the D-contraction is on partitions everywhere
  - QT/KT produced as [256ch, T] (head_dim on partitions) -> scores
    computed directly as P^T tiles [k_chunk=128, q_block=512]
  - softmax without max subtraction (scores are O(+-6); exp is fp32-safe),
    denominator = ones-column appended to V so P.V and sum(P) come out of
    one accumulation; normalize after P.V (64 rows instead of 2048)
  - causal: above-diagonal (k>q) tiles skipped entirely, diagonal tiles
    masked with affine_select after exp
"""

import sys
import types

if "/opt/trn_rl_repo" not in sys.path:
    sys.path.insert(0, "/opt/trn_rl_repo")

import numpy as np

B, T, D = 2, 2048, 1024
H, HD = 16, 64
NCORES = 8
GROUPS = 4            # head groups (cores per batch)
HPC = H // GROUPS     # heads per core = 4
CH = HPC * HD         # channels per core = 256

NDC = D // 128        # 8   d-chunks (contraction for projections)
NCC = CH // 128       # 2   channel chunks
NQB = T // 512        # 4   query blocks
NKC = T // 128        # 16  key chunks
NTC = T // 128        # 16  token chunks
NEB = D // 512        # 2   embed blocks (output projection)


def _install_axon_ntff_hook():
    """Inject the missing antenv.axon_hooks module so NTFF profiling
    (trace=True) works in this container. Harmless if never used."""
    if "antenv.axon_hooks" in sys.modules:
        return
    try:
        import antenv  # noqa: F401
    except ImportError:
        return
    mod = types.ModuleType("antenv.axon_hooks")
    mod._hook = None

    def _set(h):
        mod._hook = h

    def _get():
        return mod._hook

    mod.set_axon_ntff_profile_hook = _set
    mod.get_axon_ntff_profile_hook = _get
    sys.modules["antenv.axon_hooks"] = mod
    try:
        from trn_agent_boot.trn_boot import _ntff_profile_via_ctypes

        _set(_ntff_profile_via_ctypes("/opt/axon/libaxon_pjrt.so"))
    except Exception:
        pass


def _patch_tile_drain():
    """This walrus build rejects >2 embedded sync waits on a single
    instruction; TileContext's exit drain can carry many. Split the extras
    onto nop instructions placed just before the drain."""
    import concourse.tile as tile

    if getattr(tile.TileContext, "_drain_split_patched", False):
        return
    import bass_rust as _br
    from concourse.vector_clock import ScopedClock as _ScopedClock

    def _split_drain_and_barrier(self, tick_clock, wait_clock):
        nc = self.nc
        drain_inst = nc.sync.drain()
        wait_clock.add_sem_waits(
            drain_inst.ins, _ScopedClock({None: tick_clock.global_clock})
        )
        si = drain_inst.ins.sync_info
        waits = list(si.on_wait) if (si is not None and si.on_wait) else []
        if len(waits) > 1:
            bb = nc.cur_bb.bb
            si.on_wait = waits[:1]
            new_insts = []
            for w in waits[1:]:
                nop = nc.sync.nop()
                nop.ins.sync_info = _br.SyncInfo(on_wait=[w], on_update=[])
                bb.instructions.remove(nop.ins)
                new_insts.append(nop.ins)
            idx = bb.instructions.index(drain_inst.ins)
            for ni in reversed(new_insts):
                bb.instructions.insert(idx, ni)

        nc.all_engine_barrier()
        assert self.sems is not None
        popped = nc._tile_sem_poison_stack.pop()
        assert popped is self._sem_poison
        nc.clear_and_free_semaphores(list(self.sems.allocated().values()))
        nc.all_engine_barrier()

    tile.TileContext._drain_and_barrier = _split_drain_and_barrier
    tile.TileContext._drain_split_patched = True


def build_nc(causal: bool, debug_dumps: bool = False):
    """Build the SPMD Bass program (identical on all 8 cores)."""
    _patch_tile_drain()
    from contextlib import ExitStack

    import concourse.bacc as bacc
    import concourse.tile as tile
    from concourse import mybir

    f32 = mybir.dt.float32
    f32r = mybir.dt.float32r
    Exp = mybir.ActivationFunctionType.Exp

    nc = bacc.Bacc("TRN2")
    xT_d = nc.dram_tensor("xT", [D, T], f32r, kind="ExternalInput")
    wq_d = nc.dram_tensor("wq", [D, CH], f32r, kind="ExternalInput")
    wk_d = nc.dram_tensor("wk", [D, CH], f32r, kind="ExternalInput")
    wv_d = nc.dram_tensor("wv", [D, CH], f32r, kind="ExternalInput")
    wo_d = nc.dram_tensor("wo", [CH, D], f32r, kind="ExternalInput")
    ones_d = nc.dram_tensor("ones", [128, NTC, HPC, 1], f32r, kind="ExternalInput")
    zeros_d = nc.dram_tensor("zeros", [1, NKC * 128], f32r, kind="ExternalInput")
    out_d = nc.dram_tensor("out", [T, D], f32, kind="ExternalOutput")

    with tile.TileContext(nc) as tc:
        with ExitStack() as ctx:
            persist = ctx.enter_context(tc.tile_pool(name="persist", bufs=1))
            mm_ps = ctx.enter_context(
                tc.tile_pool(name="mm_ps", bufs=2, space="PSUM")
            )
            s_ps = ctx.enter_context(tc.tile_pool(name="s_ps", bufs=2, space="PSUM"))
            pv_ps = ctx.enter_context(tc.tile_pool(name="pv_ps", bufs=2, space="PSUM"))
            p_pool = ctx.enter_context(tc.tile_pool(name="p_pool", bufs=6))
            dn_pool = ctx.enter_context(tc.tile_pool(name="dn_pool", bufs=2))
            rc_pool = ctx.enter_context(tc.tile_pool(name="rc_pool", bufs=2))
            pvs_pool = ctx.enter_context(tc.tile_pool(name="pvs_pool", bufs=6))
            ob_pool = ctx.enter_context(tc.tile_pool(name="ob_pool", bufs=3))

            # ---- persistent SBUF tensors ----
            xT_pool = ctx.enter_context(tc.tile_pool(name="xT_pool", bufs=2))
            wq_sb = persist.tile([128, NDC, CH], f32r, tag="wq")      # 1 MB
            wk_sb = persist.tile([128, NDC, CH], f32r, tag="wk")
            wv_sb = persist.tile([128, NDC, CH], f32r, tag="wv")
            wo_sb = persist.tile([128, NCC, D], f32r, tag="wo")       # 1 MB
            # Q^T pair tiles: [2-head channel(128), head-pair, t]
            QT_sb = persist.tile([128, NCC, T], f32r, tag="QT")       # 2 MB
            # per-head zero-padded K^T score tiles [128, head, kchunk, 128]:
            # the head's 64 channels sit in its pair-partition band, the
            # other band is zero, so K=128 scores matmuls can consume the
            # full 2-head QT moving tile at full PE rate.
            KT_bd = persist.tile([128, HPC, NKC, 128], f32r, tag="KTbd")  # 4 MB
            # V with a ones-column appended per head: [t, kc, h, 65]
            V_sb = persist.tile([128, NTC, HPC, HD + 1], f32r, tag="V")
            AT_sb = persist.tile([128, NCC, T], f32r, tag="AT")
            # ---- input DMAs: the first projection needs xT(tb0) + wq,
            # so those are issued first; everything else after ----
            xT_tiles = {}
            xT0 = xT_pool.tile([128, NDC, 512], f32r, tag="xTb")
            xT_tiles[0] = xT0
            for dc in range(NDC):
                nc.sync.dma_start(
                    xT0[:, dc, :], xT_d[dc * 128 : (dc + 1) * 128, 0:512]
                )
            nc.sync.dma_start(
                wq_sb[:], wq_d.rearrange("(dc p) c -> p dc c", p=128)
            )
            xT1 = xT_pool.tile([128, NDC, 512], f32r, tag="xTb")
            xT_tiles[1] = xT1
            for dc in range(NDC):
                nc.sync.dma_start(
                    xT1[:, dc, :], xT_d[dc * 128 : (dc + 1) * 128, 512:1024]
                )
            nc.sync.dma_start(
                wk_sb[:], wk_d.rearrange("(dc p) c -> p dc c", p=128)
            )
            nc.sync.dma_start(
                wv_sb[:], wv_d.rearrange("(dc p) c -> p dc c", p=128)
            )
            nc.sync.dma_start(
                wo_sb[:], wo_d.rearrange("(cc p) e -> p cc e", p=128)
            )
            # ones columns of V (softmax denominator trick)
            nc.sync.dma_start(V_sb[:, :, :, HD : HD + 1], ones_d[:])
            # zero the unused partition band of each head's score tiles:
            # even heads use partitions 0-63 (data), 64-127 stay zero;
            # odd heads the other way around.
            for h in range(HPC):
                if h % 2 == 0:
                    zsl = KT_bd[64:128, h].rearrange("p a b -> p (a b)")
                else:
                    zsl = KT_bd[0:64, h].rearrange("p a b -> p (a b)")
                nc.sync.dma_start(
                    zsl, zeros_d[:].to_broadcast([64, NKC * 128])
                )
            # 0/1 causal masks for the two diagonal-pair patterns, used by
            # the DVE mask path (gpsimd affine_select handles the others)
            maskm = persist.tile([128, 2, 1024], f32, tag="maskm")
            nc.vector.memset(maskm[:], 1.0)
            for i in range(2):
                nc.gpsimd.affine_select(
                    out=maskm[:, i, :].rearrange("p (a b) -> p a b", a=2),
                    in_=maskm[:, i, :].rearrange("p (a b) -> p a b", a=2),
                    compare_op=mybir.AluOpType.is_ge,
                    fill=0.0,
                    base=-256 * i,
                    pattern=[[-128, 2], [1, 512]],
                    channel_multiplier=-1,
                )

            # ---- stage A: projections, per 512-token block ----
            for tb in range(NQB):
                tsl = slice(tb * 512, (tb + 1) * 512)
                if tb in xT_tiles:
                    xT_sb = xT_tiles.pop(tb)
                else:
                    xT_sb = xT_pool.tile([128, NDC, 512], f32r, tag="xTb")
                    for dc in range(NDC):
                        nc.sync.dma_start(
                            xT_sb[:, dc, :],
                            xT_d[dc * 128 : (dc + 1) * 128, tsl],
                        )
                for cc in range(NCC):
                    # Q projection (2 heads per pair tile)
                    ps = mm_ps.tile([128, 512], f32, tag="mmps")
                    for dc in range(NDC):
                        nc.tensor.matmul(
                            ps[:],
                            wq_sb[:, dc, cc * 128 : (cc + 1) * 128],
                            xT_sb[:, dc, :],
                            start=(dc == 0),
                            stop=(dc == NDC - 1),
                        )
                    nc.vector.tensor_copy(QT_sb[:, cc, tsl], ps[:])
                    # K projection -> per-head zero-padded score tiles
                    psk = mm_ps.tile([128, 512], f32, tag="mmps")
                    for dc in range(NDC):
                        nc.tensor.matmul(
                            psk[:],
                            wk_sb[:, dc, cc * 128 : (cc + 1) * 128],
                            xT_sb[:, dc, :],
                            start=(dc == 0),
                            stop=(dc == NDC - 1),
                        )
                    for kcl in range(4):
                        kc = tb * 4 + kcl
                        off = kcl * 128
                        h0, h1 = 2 * cc, 2 * cc + 1
                        nc.vector.tensor_copy(
                            KT_bd[0:64, h0, kc, :], psk[0:64, off : off + 128]
                        )
                        nc.vector.tensor_copy(
                            KT_bd[64:128, h1, kc, :],
                            psk[64:128, off : off + 128],
                        )
                for tci in range(4):
                    t_c = tb * 4 + tci
                    psv = mm_ps.tile([128, 512], f32, tag="mmps")
                    for dc in range(NDC):
                        nc.tensor.matmul(
                            psv[:, 0:CH],
                            xT_sb[:, dc, tci * 128 : (tci + 1) * 128],
                            wv_sb[:, dc, :],
                            start=(dc == 0),
                            stop=(dc == NDC - 1),
                        )
                    nc.vector.tensor_copy(
                        V_sb[:, t_c, :, 0:HD],
                        psv[:, 0:CH].rearrange("p (h d) -> p h d", h=HPC),
                    )

            # ---- stage B: attention, per (head, query block) ----
            # All matmuls are K=128 (full PE rate; K-dim switches between
            # shapes cost ~2x). Scores tiles are emitted in pairs into one
            # 2-bank psum tile and exp'd with a single [128,1024] ACT op;
            # the PV accumulation of pair j-1 is interleaved after the
            # scores of pair j so the PE never waits on the ACT.
            def emit_pair(h, qb, j):
                qsl = slice(qb * 512, (qb + 1) * 512)
                s = s_ps.tile([128, 1024], f32, tag="s")
                for half in (0, 1):
                    kc = 2 * j + half
                    nc.tensor.matmul(
                        s[:, half * 512 : (half + 1) * 512],
                        KT_bd[:, h, kc, :],
                        QT_sb[:, h // 2, qsl],
                        start=True,
                        stop=True,
                    )
                p = p_pool.tile([128, 1024], f32r, tag="p")
                nc.scalar.activation(p[:], s[:], Exp)
                if causal and 2 * j >= 4 * qb - 1:
                    # diagonal pair: mask both halves in ONE op; alternate
                    # engines so the mask never paces the pipeline.
                    pat = 0 if 2 * j == 4 * qb else 1
                    state["mask_flip"] = not state.get("mask_flip", False)
                    if state["mask_flip"]:
                        nc.vector.tensor_mul(p[:], p[:], maskm[:, pat, :])
                    else:
                        nc.gpsimd.affine_select(
                            out=p[:].rearrange("p (a b) -> p a b", a=2),
                            in_=p[:].rearrange("p (a b) -> p a b", a=2),
                            compare_op=mybir.AluOpType.is_ge,
                            fill=0.0,
                            base=qb * 512 - 2 * j * 128,
                            pattern=[[-128, 2], [1, 512]],
                            channel_multiplier=-1,
                        )
                return p

            # Flat cross-block pipeline over all (qb, h) attention blocks:
            # each block's first scores pair is emitted during the previous
            # block's PV drain, so the PE never waits on the exp+mask chain
            # at block boundaries. Each qb's normalizations are emitted when
            # its 4 blocks finish; its output projection is delayed by one
            # block so the norm chain (slow DVE reciprocal) completes first.
            def emit_norms(qb):
                qsl = slice(qb * 512, (qb + 1) * 512)
                for h, pvs in pvs_lists[qb]:
                    hp, hoi = h // 2, h % 2
                    ho = 64 * hoi
                    rc = rc_pool.tile([1, 512], f32, tag="rc")
                    nc.vector.reciprocal(rc[:], pvs[HD : HD + 1, :])
                    dn = dn_pool.tile([64, 512], f32, tag="dn")
                    nc.gpsimd.partition_broadcast(dn[:], rc[:])
                    nc.vector.tensor_mul(
                        AT_sb[ho : ho + 64, hp, qsl], pvs[0:HD, :], dn[:]
                    )

            def emit_stagec(qb):
                for t_c in range(qb * 4, (qb + 1) * 4):
                    for eb in range(NEB):
                        esl = slice(eb * 512, (eb + 1) * 512)
                        ps = mm_ps.tile([128, 512], f32, tag="mmps")
                        for cc in range(NCC):
                            nc.tensor.matmul(
                                ps[:],
                                AT_sb[:, cc, t_c * 128 : (t_c + 1) * 128],
                                wo_sb[:, cc, esl],
                                start=(cc == 0),
                                stop=(cc == NCC - 1),
                            )
                        ob = ob_pool.tile([128, 512], f32, tag="ob")
                        nc.scalar.copy(ob[:], ps[:])
                        nc.sync.dma_start(
                            out_d[t_c * 128 : (t_c + 1) * 128, esl], ob[:]
                        )

            blocks = []
            for qb in range(NQB):
                nkc = 4 * (qb + 1) if causal else NKC
                for h in range(HPC):
                    blocks.append((qb, h, nkc // 2))
            pvs_lists = {qb: [] for qb in range(NQB)}
            # PV lags the scores stream by TWO pairs so the PE's cover work
            # (~2 scores pairs) always exceeds the exp+mask chain latency.
            state = {"pend_pv": [], "pending_c": []}

            def pop_pv():
                qb, h, j, last, p_tile, pv = state["pend_pv"].pop(0)
                for half in (0, 1):
                    kc = 2 * j + half
                    nc.tensor.matmul(
                        pv[:],
                        V_sb[:, kc, h, :],
                        p_tile[:, half * 512 : (half + 1) * 512],
                        start=(kc == 0),
                        stop=(last and half == 1),
                    )
                if last:
                    pvs = pvs_pool.tile([HD + 1, 512], f32, tag="pvs")
                    nc.scalar.copy(pvs[:], pv[:])
                    pvs_lists[qb].append((h, pvs))
                    if len(pvs_lists[qb]) == HPC:
                        emit_norms(qb)
                        state["pending_c"].append(qb)

            for bi, (qb, h, npair) in enumerate(blocks):
                pv = pv_ps.tile([HD + 1, 512], f32, tag="pv")
                for j in range(npair):
                    p = emit_pair(h, qb, j)
                    had_pending = list(state["pending_c"])
                    state["pend_pv"].append(
                        (qb, h, j, j == npair - 1, p, pv)
                    )
                    if len(state["pend_pv"]) > 2:
                        pop_pv()
                    # emit a deferred output projection one block late, right
                    # after this block's first scores pair is in flight
                    if j == 0 and had_pending:
                        emit_stagec(state["pending_c"].pop(0))
            while state["pend_pv"]:
                pop_pv()
            for qb in state["pending_c"]:
                emit_stagec(qb)

            if debug_dumps:
                dA = nc.dram_tensor(
                    "dbg_AT", [128, NCC, T], f32r, kind="ExternalOutput"
                )
                nc.sync.dma_start(dA[:], AT_sb[:])
                dQ = nc.dram_tensor(
                    "dbg_QT", [128, NCC, T], f32r, kind="ExternalOutput"
                )
                nc.sync.dma_start(dQ[:], QT_sb[:])
                dK = nc.dram_tensor(
                    "dbg_KT", [128, HPC, NKC, 128], f32r, kind="ExternalOutput"
                )
                nc.sync.dma_start(dK[:], KT_bd[:])
                dV = nc.dram_tensor(
                    "dbg_V", [128, NTC, HPC, HD + 1], f32r, kind="ExternalOutput"
                )
                nc.sync.dma_start(dV[:], V_sb[:])

    nc.finalize()
    return nc


def make_in_maps(q_input, wq, wk, wv, wo):
    q_input = np.asarray(q_input, dtype=np.float32)
    wq = np.asarray(wq, dtype=np.float32)
    wk = np.asarray(wk, dtype=np.float32)
    wv = np.asarray(wv, dtype=np.float32)
    wo = np.asarray(wo, dtype=np.float32)
    scale = 1.0 / np.sqrt(np.float32(HD))

    in_maps = []
    for core in range(NCORES):
        b, g = divmod(core, GROUPS)
        G = slice(g * CH, (g + 1) * CH)
        in_maps.append(
            {
                "xT": np.ascontiguousarray(q_input[b].T),
                "wq": np.ascontiguousarray(wq[G, :].T * scale),
                "wk": np.ascontiguousarray(wk[G, :].T),
                "wv": np.ascontiguousarray(wv[G, :].T),
                "wo": np.ascontiguousarray(wo[:, G].T),
                "ones": np.ones((128, NTC, HPC, 1), np.float32),
                "zeros": np.zeros((1, NKC * 128), np.float32),
            }
        )
    return in_maps


def _gather(results, bo):
    out = np.zeros((B, T, D), np.float32)
    for core in range(NCORES):
        out[core // GROUPS] += results[core]["out"]
    out += np.asarray(bo, dtype=np.float32)
    return out


def _run(q_input, wq, wk, wv, wo, bo, mask, trace=False, trace_kwargs=None):
    _install_axon_ntff_hook()
    from concourse.bass_utils import run_bass_kernel_spmd

    causal = bool(np.asarray(mask).item()) if not isinstance(mask, int) else bool(mask)
    nc = build_nc(causal)
    in_maps = make_in_maps(q_input, wq, wk, wv, wo)
    res = run_bass_kernel_spmd(
        nc,
        in_maps,
        list(range(NCORES)),
        trace=trace,
        **(trace_kwargs or {}),
    )
    return _gather(res.results, bo), res


def kernel(q_input, wq, wk, wv, wo, bo, mask):
    out, _ = _run(q_input, wq, wk, wv, wo, bo, mask)
    return out

